# revision 34
# baseline (speedup 1.0000x reference)
"""Tensor-parallel GQA attention block on 8 TRN2 NeuronCores (Bass/Tile).

Problem: B=1, S=2048, DIM=4096, 32 q heads / 8 kv heads (GQA), head_dim=128,
RoPE, causal softmax, output projection.

Sharding (tensor parallel by head, per the hint): core c of 8 owns q heads
4c..4c+3 and kv head c (GQA groups stay with their q heads). wqkv rows and wo
columns are sharded by head; attention is fully local per core; each core
emits a partial (S, DIM) output (its heads through its wo column slice) and
the partials are summed on the host at unshard time (the "all-reduce after
wo" of the hint, done off-device since full I/O passes through the host
anyway).

Per-core device kernel -- all operands host-pre-transposed so every matmul has
its contraction dim on SBUF partitions; zero on-device transposes:
  qkT = wqkT.T @ xT              (head dims on partitions, seq free)
  v   = xT.T @ wvT               (seq on partitions, head dim free)
  RoPE on qT/kT in transposed layout: host permutes rows into re(0..63)/
    im(64..127); cos/sin arrive as stacked (128, S) tables [cos;cos] and
    [-sin;sin]; 1/sqrt(HD) is folded into wq on the host.
  per head, per 512-wide q chunk (causal: only k tiles <= chunk end):
    S.T[j] = kT_j.T @ qT_chunk   (k positions on partitions => softmax
                                  denominators via a ones-matmul; no P
                                  transpose anywhere)
    P.T[j] = exp(S.T[j] - 12)    (triangular mask added on diagonal tiles;
                                  N trimmed to the causal columns)
    sums  += ones128.T @ P.T[j]  (PSUM-accumulated, rows replicated)
    O.T   += matmul(lhsT=V_j, rhs=P.T[j])
    O.T_norm = O.T * reciprocal_approx(sums)  -> bf16
  out[t, d] = sum_h O.T_h[:, t].T @ woT_h[:, d]

Compute in bf16 with f32 PSUM accumulation; rel l2 error vs the f32 reference
is ~8e-3.  Performance structure: phases are emitted in [A A C B] windows so
the attention exp stream (the PE's only tight cross-engine dependency) is
never queued on ACT behind bulk copy work: output-projection PSUM->SBUF
copies alternate DVE/ACT and sit in the C slot where ACT is otherwise idle,
RoPE swap copies run on ACT during the A slots, exps run alone during B.
x streams in 512-wide tiles shared by the two half-chunk A phases; weights
stream in 4-k-tile groups on the SWDGE queue while x uses the HWDGE queue.
"""
import sys

sys.path.insert(0, "/opt/trn_rl_repo")

from contextlib import ExitStack

import numpy as np
import ml_dtypes

import concourse.bass as bass
import concourse.tile as tile
import concourse.mybir as mybir
from concourse import bacc
from concourse.bass_utils import run_bass_kernel_spmd

F32 = mybir.dt.float32
BF16 = mybir.dt.bfloat16
NPBF16 = ml_dtypes.bfloat16

NH, NKV, HD = 32, 8, 128
S, DIM = 2048, 4096
N_CORES = 8
NHL = NH // N_CORES          # q heads per core
PERM = np.concatenate([np.arange(0, 128, 2), np.arange(1, 128, 2)])


def build_attention_kernel(nc, S=2048, DIM=4096, C=12.0):
    NHL = 4          # local q heads
    HD = 128
    CHUNK = 512
    P = 128
    NKT = DIM // P         # k tiles over model dim
    NCH = S // CHUNK       # seq chunks
    QKM = NHL + 1          # m-tiles in qk GEMM (4 q heads + 1 k head)
    NDC = DIM // CHUNK     # output dim chunks

    # ---- DRAM I/O ----
    # x arrives chunk-major from the host: xG[p, ch, g, j, s] = x[ch*512+s,
    # (4g+j)*128+p], so one DMA pulls 4 k-tiles of one seq chunk as a single
    # contiguous 4KB run per partition (one descriptor per partition).
    xG = nc.dram_tensor("xG", (128, S // 512, DIM // 512, 4, 512), BF16,
                        kind="ExternalInput").ap()
    wqkT = nc.dram_tensor("wqkT", (DIM, QKM * P), BF16, kind="ExternalInput").ap()
    wvT = nc.dram_tensor("wvT", (DIM, HD), BF16, kind="ExternalInput").ap()
    woT = nc.dram_tensor("woT", (NHL * HD, DIM), BF16, kind="ExternalInput").ap()
    # cosX rows 0-63 and 64-127 both hold cos; sinX rows 0-63 hold -sin,
    # rows 64-127 hold +sin (see host prep) -- lets RoPE run as 3 full-width
    # DVE ops with matching base partitions.  bf16 (rotation error ~0.2%)
    # to free SBUF for the softmax-sum accumulators.
    cosT = nc.dram_tensor("cosT", (128, S), BF16, kind="ExternalInput").ap()
    sinT = nc.dram_tensor("sinT", (128, S), BF16, kind="ExternalInput").ap()
    onesW = nc.dram_tensor("onesW", (P, P), BF16, kind="ExternalInput").ap()
    maskT = nc.dram_tensor("maskT", (P, P), F32, kind="ExternalInput").ap()
    out = nc.dram_tensor("out", (S, DIM), BF16, kind="ExternalOutput").ap()

    with tile.TileContext(nc) as tc, ExitStack() as ctx:
        const = ctx.enter_context(tc.tile_pool(name="const", bufs=1))
        resid = ctx.enter_context(tc.tile_pool(name="resid", bufs=1))
        xpool = ctx.enter_context(tc.tile_pool(name="xp", bufs=10))
        ptpool = ctx.enter_context(tc.tile_pool(name="ptp", bufs=12))
        tmppool = ctx.enter_context(tc.tile_pool(name="tmp", bufs=4))
        obpool = ctx.enter_context(tc.tile_pool(name="obp", bufs=5))
        psum = ctx.enter_context(tc.tile_pool(name="psum", bufs=8, space="PSUM"))

        # ---- startup streams: A0's diet (x chunk 0 + wqk + wv, 8.2MB) is
        # HBM-bound against A0's ~21us of PE work, so it is split across the
        # two DMA queues in the PE's k-group consumption order: even wqk
        # groups + all wv on the SWDGE queue, odd wqk groups (but g7)
        # interleaved with the x groups on the HWDGE queue. ----
        NKG = NKT // 4
        wqk_g = [const.tile([P, 4, QKM * P], BF16, tag=f"wqkg{g}", name=f"wqkg{g}")
                 for g in range(NKG)]
        wv_g = [const.tile([P, 4, HD], BF16, tag=f"wvg{g}", name=f"wvg{g}")
                for g in range(NKG)]

        def dma_wqk(g, eng):
            if g == 0:
                # per-k-tile DMAs for the first group: the first matmul can
                # start after one 160KB k-tile instead of the 640KB group.
                for j in range(4):
                    eng.dma_start(wqk_g[0][:, j],
                                  wqkT[j * P:(j + 1) * P, :])
                return
            eng.dma_start(
                wqk_g[g][:],
                wqkT[g * 4 * P:(g + 1) * 4 * P, :].rearrange(
                    "(kt p) m -> p kt m", p=P))

        def dma_wv(g, eng):
            eng.dma_start(
                wv_g[g][:],
                wvT[g * 4 * P:(g + 1) * 4 * P, :].rearrange(
                    "(kt p) m -> p kt m", p=P))

        x_ch = {}

        def load_x_chunk(ch, interleave=()):
            xgs = []
            for g in range(NKT // 4):
                xg = xpool.tile([P, 4, CHUNK], BF16, tag="xt", name="xt")
                nc.sync.dma_start(xg[:], xG[:, ch, g])
                xgs.append(xg)
                for wg in interleave:
                    if wg[0] == g:
                        dma_wqk(wg[1], nc.sync)
            x_ch[ch] = [xgs[k // 4][:, k % 4] for k in range(NKT)]

        for g in range(NKG):
            dma_wqk(g, nc.gpsimd)
            dma_wv(g, nc.gpsimd)
        wqk_sb = [wqk_g[k // 4][:, k % 4] for k in range(NKT)]
        wv_sb = [wv_g[k // 4][:, k % 4] for k in range(NKT)]
        ones_sb = const.tile([P, P], BF16, tag="ones", name="ones")
        nc.gpsimd.dma_start(ones_sb[:], onesW[:])
        mask_sb = const.tile([P, P], F32, tag="mask", name="mask")
        nc.gpsimd.dma_start(mask_sb[:], maskT[:])
        negC = const.tile([P, 1], F32, tag="negC", name="negC")
        nc.any.memset(negC[:], -C)
        wo_sb = const.tile([P, NHL, DIM], BF16, tag="wo", name="wo")
        nc.gpsimd.dma_start(wo_sb[:], woT.rearrange("(h p) n -> p h n", p=P))

        load_x_chunk(0)
        cos_sb = const.tile([P, S], BF16, tag="cos", name="cos")
        nc.sync.dma_start(cos_sb[:], cosT[:])
        sin_sb = const.tile([P, S], BF16, tag="sin", name="sin")
        nc.sync.dma_start(sin_sb[:], sinT[:])

        # resident activations (per chunk tiles for fine-grained deps)
        q_sb = [[resid.tile([P, CHUNK], BF16, tag=f"q{h}_{ch}", name=f"q{h}_{ch}")
                 for ch in range(NCH)] for h in range(NHL)]
        k_sb = [resid.tile([P, CHUNK], BF16, tag=f"k{ch}", name=f"k{ch}")
                for ch in range(NCH)]
        v_sb = [resid.tile([P, CHUNK], BF16, tag=f"v{ch}", name=f"v{ch}")
                for ch in range(NCH)]
        ot_sb = [[resid.tile([P, CHUNK], BF16, tag=f"ot{h}_{ch}", name=f"ot{h}_{ch}")
                  for ch in range(NCH)] for h in range(NHL)]

        def rope_hc(ps, raw_sw, out_tile, hc):
            """ps: (128, CHUNK//2) f32 PSUM [re; im]; raw_sw: bf16 SBUF with
            halves swapped [im; re] (produced by two ACT copies).
            out = ps*cosX + raw_sw*sinX with cosX = [cos; cos],
            sinX = [-sin; +sin]:
              out[0:64]   = re*cos - im*sin
              out[64:128] = im*cos + re*sin
            The cos mul reads PSUM directly (mixed PSUM+SBUF operands are
            exempt from the matching-base-partition rule); the ACT swap copies
            plus one DVE mul free the bank quickly while the remaining DVE ops
            run off-PSUM."""
            HC2 = CHUNK // 2
            cos = cos_sb[:, hc * HC2:(hc + 1) * HC2]
            sin = sin_sb[:, hc * HC2:(hc + 1) * HC2]
            t1 = tmppool.tile([P, HC2], F32, tag="t1", name="t1", bufs=2)
            t2 = tmppool.tile([P, HC2], F32, tag="t2", name="t2", bufs=2)
            nc.vector.tensor_mul(t1[:], ps[:], cos)
            nc.vector.tensor_mul(t2[:], raw_sw[:], sin)
            nc.vector.tensor_add(out_tile[:], t1[:], t2[:])

        HC = CHUNK // 2      # 256-wide half chunks: the qk PSUM footprint
        # drops to 3 banks (two heads packed per bank) + 1 shared V bank, so
        # phases B/C always find free banks and never stall on A's epilogue.
        vbank = [None]
        a_state = {}

        def phase_a(hc, skip_ep=False):
            """qkv projection (+ RoPE epilogue unless skip_ep) for half-chunk
            hc.  skip_ep defers the ACT/DVE epilogue emission (phase_a_ep) so
            another phase's exps can jump ahead of it in the ACT queue."""
            ch, half = hc // 2, hc % 2
            qk_bank = [psum.tile([P, CHUNK], F32, tag="ps", name="ps")
                       for _ in range((QKM + 1) // 2)]
            if half == 0:
                vbank[0] = psum.tile([P, CHUNK], F32, tag="ps", name="ps")
                if ch not in x_ch:
                    load_x_chunk(ch)
            ps_v = vbank[0]
            a_state[hc] = (qk_bank, ps_v)

            def qk_slice(m):
                return qk_bank[m // 2][:, (m % 2) * HC:(m % 2 + 1) * HC]

            for k in range(NKT):
                xt = x_ch[ch][k]
                xh = xt[:, half * HC:(half + 1) * HC]
                for m in range(QKM):
                    nc.tensor.matmul(
                        qk_slice(m), wqk_sb[k][:, m * P:(m + 1) * P], xh,
                        start=(k == 0 and m % 2 == 0),
                        stop=(k == NKT - 1 and (m % 2 == 1 or m == QKM - 1)),
                        skip_group_check=True)
                for t in range(2):
                    tt = 2 * half + t
                    nc.tensor.matmul(
                        ps_v[:, tt * P:(tt + 1) * P],
                        xt[:, tt * P:(tt + 1) * P], wv_sb[k][:],
                        start=(half == 0 and k == 0 and t == 0),
                        stop=(half == 1 and k == NKT - 1 and t == 1),
                        skip_group_check=True)
            if half == 1:
                del x_ch[ch]
            if not skip_ep:
                phase_a_ep(hc)

        def phase_a_ep(hc):
            """v copy + RoPE swap copies (ACT) and RoPE muls (DVE) for hc."""
            ch, half = hc // 2, hc % 2
            qk_bank, ps_v = a_state.pop(hc)

            def qk_slice(m):
                return qk_bank[m // 2][:, (m % 2) * HC:(m % 2 + 1) * HC]

            if half == 1:
                nc.scalar.copy(v_sb[ch][:], ps_v[:])
            rawsw = [tmppool.tile([P, HC], BF16, tag=f"qksw{m}", name=f"qksw{m}", bufs=2)
                     for m in range(QKM)]
            order = list(range(NHL)) + [NHL]     # banks free in alloc order
            for m in order:
                nc.scalar.copy(rawsw[m][0:64, :], qk_slice(m)[64:128, :])
                nc.scalar.copy(rawsw[m][64:128, :], qk_slice(m)[0:64, :])
            for m in order:
                out_tile = k_sb[ch] if m == NHL else q_sb[m][ch]
                rope_hc(qk_slice(m), rawsw[m],
                        out_tile[:, half * HC:(half + 1) * HC], hc)

        def score(ch, h, j):
            """score matmul + mask + exp for k-tile j of (head h, chunk ch)."""
            o = j - 4 * ch          # >=0: diagonal region, trim N
            lo = max(o, 0) * P      # first valid q column
            ps_st = psum.tile([P, CHUNK], F32, tag="ps", name="ps")
            nc.tensor.matmul(
                ps_st[:, lo:], k_sb[j // 4][:, (j % 4) * P:(j % 4 + 1) * P],
                q_sb[h][ch][:, lo:], start=True, stop=True)
            pt = ptpool.tile([P, CHUNK], BF16, tag="pt", name="pt")
            if o >= 0:  # mask the diagonal 128x128 block
                nc.vector.tensor_add(
                    ps_st[:, o * P:(o + 1) * P],
                    ps_st[:, o * P:(o + 1) * P], mask_sb[:])
            nc.scalar.activation(
                pt[:, lo:], ps_st[:, lo:],
                mybir.ActivationFunctionType.Exp, bias=negC[:])
            return pt, lo

        prescored = {}

        def finalize(ch, h, acc, ps_ot):
            """softmax denominators from the DVE-accumulated partial sums:
            one bf16 downcast + one 512-col ones-matmul per (head, chunk)
            replaces the per-k-tile ones-matmuls (saves ~25us of PE rows)."""
            acc_bf = tmppool.tile([P, CHUNK], BF16, tag="accbf", name="accbf",
                                  bufs=2)
            nc.vector.tensor_copy(out=acc_bf[:], in_=acc[:])
            ps_sum = psum.tile([P, CHUNK], F32, tag="ps", name="ps")
            nc.tensor.matmul(ps_sum[:], ones_sb[:], acc_bf[:],
                             start=True, stop=True)
            recip = tmppool.tile([P, CHUNK], F32, tag="recip", name="recip",
                                 bufs=2)
            nc.vector.reciprocal_approx_fast(out=recip[:], in_=ps_sum[:])
            nc.vector.tensor_mul(ot_sb[h][ch][:], ps_ot[:], recip[:])

        def phase_b(ch):
            """attention for all local heads, q chunk ch (causal).
            Software-pipelined: scores run 2 ahead of the exp-dependent PV
            matmuls.  Heads 0..2 accumulate pt into acc on the DVE (partition
            sums via one tiny matmul in finalize, emitted one head late so
            the DVE chain can drain under the next head's PE work); the last
            head keeps the classic per-k-tile ones-matmul so nothing ever
            defers past the end of the phase."""
            njt = 4 * ch + 4
            fin = None
            for h in range(NHL):
                classic = h == NHL - 1
                ps_ot = psum.tile([P, CHUNK], F32, tag="ps", name="ps")
                if classic:
                    ps_sum = psum.tile([P, CHUNK], F32, tag="ps", name="ps")
                else:
                    acc = tmppool.tile([P, CHUNK], F32, tag="acc", name="acc",
                                       bufs=2)
                if h == 0 and ch in prescored:
                    nxt = prescored.pop(ch)
                else:
                    nxt = [score(ch, h, 0), score(ch, h, 1)]
                for j in range(njt):
                    pt, lo = nxt.pop(0)
                    if j + 2 < njt:
                        nxt.append(score(ch, h, j + 2))
                    if classic:
                        nc.tensor.matmul(ps_sum[:, lo:], ones_sb[:],
                                         pt[:, lo:], start=(j == 0),
                                         stop=(j == njt - 1))
                    elif j == 0:
                        nc.vector.tensor_copy(out=acc[:], in_=pt[:])
                    else:
                        nc.vector.tensor_add(acc[:, lo:], acc[:, lo:],
                                             pt[:, lo:])
                    nc.tensor.matmul(
                        ps_ot[:, lo:], v_sb[j // 4][:, (j % 4) * P:(j % 4 + 1) * P],
                        pt[:, lo:], start=(j == 0), stop=(j == njt - 1))
                if fin is not None:
                    finalize(*fin)
                    fin = None
                if classic:
                    recip = tmppool.tile([P, CHUNK], F32, tag="recip",
                                         name="recip", bufs=2)
                    nc.vector.reciprocal_approx_fast(out=recip[:],
                                                     in_=ps_sum[:])
                    nc.vector.tensor_mul(ot_sb[h][ch][:], ps_ot[:], recip[:])
                else:
                    fin = (ch, h, acc, ps_ot)

        def phase_c(ch, pre_ch=None):
            """output projection for the 4 seq tiles of chunk ch.  The
            PSUM->SBUF bf16 copies alternate DVE/ACT so neither engine queues
            more than half of them ahead of the next B phase's exps.  pre_ch:
            emit the next B phase's first two scores+exps before the last
            quarter, hiding the exp latency at the B start."""
            for tq in range(4):
                if tq == 3 and pre_ch is not None:
                    prescored[pre_ch] = [score(pre_ch, 0, 0),
                                         score(pre_ch, 0, 1)]
                t = 4 * ch + tq
                for d in range(NDC):
                    ps_o = psum.tile([P, CHUNK], F32, tag="ps", name="ps")
                    for h in range(NHL):
                        nc.tensor.matmul(
                            ps_o[:], ot_sb[h][ch][:, tq * P:(tq + 1) * P],
                            wo_sb[:, h, d * CHUNK:(d + 1) * CHUNK],
                            start=(h == 0), stop=(h == NHL - 1))
                    ob = obpool.tile([P, CHUNK], BF16, tag="ob", name="ob")
                    if d % 2 == 0:
                        nc.vector.tensor_copy(out=ob[:], in_=ps_o[:])
                    else:
                        nc.scalar.copy(ob[:], ps_o[:])
                    oeng = nc.sync if d % 2 == 0 else nc.gpsimd
                    oeng.dma_start(
                        out[t * P:(t + 1) * P, d * CHUNK:(d + 1) * CHUNK], ob[:])

        # [A A C B] windows: C's PE work separates the A pair's ACT swap
        # copies (and C's own alternating copies) from B's exp stream, so by
        # the time the PE reaches B's matmuls ACT is drained and exps issue
        # back-to-back.  B0 (the one B with no C before it) eats a one-time
        # ~4us exp wait behind A3's swap copies; deferring those copies past
        # B0 would deadlock the shared 8-bank PSUM ring, so it stays.
        phase_a(0)
        phase_a(1)
        phase_a(2)
        phase_a(3)
        phase_b(0)
        for ch in range(2, NCH):
            phase_a(2 * ch)
            phase_a(2 * ch + 1)
            phase_c(ch - 2, pre_ch=ch - 1)
            phase_b(ch - 1)
        phase_c(NCH - 2, pre_ch=NCH - 1)
        phase_b(NCH - 1)
        phase_c(NCH - 1)

    return nc


def _make_in_maps(x, freqs_cis, wqkv, wo):
    scale = np.float32(1.0 / np.sqrt(HD))
    # chunk-major x: xG[p, ch, g, j, s] = x[0, ch*512+s, (4g+j)*128+p], so a
    # (ch, g) slice is one contiguous 4KB run per partition.
    xG = np.ascontiguousarray(
        np.asarray(x)[0].reshape(S // 512, 512, DIM // 512, 4, 128)
        .transpose(4, 0, 2, 3, 1)).astype(NPBF16)
    cos = freqs_cis[:, :, 0].T.astype(np.float32)        # (64, S)
    sin = freqs_cis[:, :, 1].T.astype(np.float32)
    cosT = np.ascontiguousarray(
        np.concatenate([cos, cos], 0)).astype(NPBF16)   # (128, S)
    sinT = np.ascontiguousarray(
        np.concatenate([-sin, sin], 0)).astype(NPBF16)
    ones = np.ones((128, 128), NPBF16)
    kp = np.arange(128)[:, None]
    qp = np.arange(128)[None, :]
    maskT = np.where(kp <= qp, 0.0, -1e30).astype(np.float32)

    in_maps = []
    for c in range(N_CORES):
        rows = [wqkv[128 * (NHL * c + h) + PERM] * scale for h in range(NHL)]
        rows.append(wqkv[NH * HD + 128 * c + PERM])
        wqkT = np.ascontiguousarray(np.concatenate(rows, 0).T).astype(NPBF16)
        wvT = np.ascontiguousarray(
            wqkv[(NH + NKV) * HD + 128 * c:(NH + NKV) * HD + 128 * (c + 1)].T
        ).astype(NPBF16)
        woT = np.ascontiguousarray(
            wo[:, 128 * NHL * c:128 * NHL * (c + 1)].T).astype(NPBF16)
        in_maps.append({
            "xG": xG, "wqkT": wqkT, "wvT": wvT, "woT": woT,
            "cosT": cosT, "sinT": sinT, "onesW": ones, "maskT": maskT,
        })
    return in_maps


def kernel(x, freqs_cis, wqkv, wo):
    x = np.asarray(x, dtype=np.float32)
    freqs_cis = np.asarray(freqs_cis, dtype=np.float32)
    wqkv = np.asarray(wqkv, dtype=np.float32)
    wo = np.asarray(wo, dtype=np.float32)

    in_maps = _make_in_maps(x, freqs_cis, wqkv, wo)
    nc = bacc.Bacc("TRN2", target_bir_lowering=False, debug=False,
                   num_devices=N_CORES)
    build_attention_kernel(nc, S=S, DIM=DIM)
    nc.compile()
    res = run_bass_kernel_spmd(nc, in_maps, core_ids=list(range(N_CORES)))

    acc = np.zeros((S, DIM), np.float32)
    for r in res.results:
        acc += np.asarray(r["out"]).astype(np.float32)
    return acc[None]


# revision 35
# speedup vs baseline: 1.1801x; 1.1801x over previous
"""Tensor-parallel GQA attention block on 8 TRN2 NeuronCores (Bass/Tile).

Problem: B=1, S=2048, DIM=4096, 32 q heads / 8 kv heads (GQA), head_dim=128,
RoPE, causal softmax, output projection.

Sharding (tensor parallel by head, per the hint): core c of 8 owns q heads
4c..4c+3 and kv head c (GQA groups stay with their q heads). wqkv rows and wo
columns are sharded by head; attention is fully local per core; each core
emits a partial (S, DIM) output (its heads through its wo column slice) and
the partials are summed on the host at unshard time (the "all-reduce after
wo" of the hint, done off-device since full I/O passes through the host
anyway).

Per-core device kernel -- all operands host-pre-transposed so every matmul has
its contraction dim on SBUF partitions; zero on-device transposes:
  qkT = wqkT.T @ xT              (head dims on partitions, seq free)
  v   = xT.T @ wvT               (seq on partitions, head dim free)
  RoPE on qT/kT in transposed layout: host permutes rows into re(0..63)/
    im(64..127); cos/sin arrive as stacked (128, S) tables [cos;cos] and
    [-sin;sin]; 1/sqrt(HD) is folded into wq on the host.
  per head, per 512-wide q chunk (causal: only k tiles <= chunk end):
    S.T[j] = kT_j.T @ qT_chunk   (k positions on partitions => softmax
                                  denominators via a ones-matmul; no P
                                  transpose anywhere)
    P.T[j] = exp(S.T[j] - 12)    (triangular mask added on diagonal tiles;
                                  N trimmed to the causal columns)
    sums  += ones128.T @ P.T[j]  (PSUM-accumulated, rows replicated)
    O.T   += matmul(lhsT=V_j, rhs=P.T[j])
    O.T_norm = O.T * reciprocal_approx(sums)  -> bf16
  out[t, d] = sum_h O.T_h[:, t].T @ woT_h[:, d]

Compute in bf16 with f32 PSUM accumulation; rel l2 error vs the f32 reference
is ~8e-3.  Performance structure: phases are emitted in [A A C B] windows so
the attention exp stream (the PE's only tight cross-engine dependency) is
never queued on ACT behind bulk copy work: output-projection PSUM->SBUF
copies alternate DVE/ACT and sit in the C slot where ACT is otherwise idle,
RoPE swap copies run on ACT during the A slots, exps run alone during B.
x streams in 512-wide tiles shared by the two half-chunk A phases; weights
stream in 4-k-tile groups on the SWDGE queue while x uses the HWDGE queue.
"""
import sys

sys.path.insert(0, "/opt/trn_rl_repo")

from contextlib import ExitStack

import numpy as np
import ml_dtypes

import concourse.bass as bass
import concourse.tile as tile
import concourse.mybir as mybir
from concourse import bacc
from concourse.bass_utils import run_bass_kernel_spmd

F32 = mybir.dt.float32
BF16 = mybir.dt.bfloat16
NPBF16 = ml_dtypes.bfloat16

NH, NKV, HD = 32, 8, 128
S, DIM = 2048, 4096
N_CORES = 8
NHL = NH // N_CORES          # q heads per core
PERM = np.concatenate([np.arange(0, 128, 2), np.arange(1, 128, 2)])


def build_attention_kernel(nc, S=2048, DIM=4096, C=12.0):
    NHL = 4          # local q heads
    HD = 128
    CHUNK = 512
    P = 128
    NKT = DIM // P         # k tiles over model dim
    NCH = S // CHUNK       # seq chunks
    QKM = NHL + 1          # m-tiles in qk GEMM (4 q heads + 1 k head)
    NDC = DIM // CHUNK     # output dim chunks

    # ---- DRAM I/O ----
    # x arrives chunk-major from the host: xG[p, ch, g, j, s] = x[ch*512+s,
    # (4g+j)*128+p], so one DMA pulls 4 k-tiles of one seq chunk as a single
    # contiguous 4KB run per partition (one descriptor per partition).
    xG = nc.dram_tensor("xG", (128, S // 512, DIM // 512, 4, 512), BF16,
                        kind="ExternalInput").ap()
    wqkT = nc.dram_tensor("wqkT", (DIM, QKM * P), BF16, kind="ExternalInput").ap()
    wvT = nc.dram_tensor("wvT", (DIM, HD), BF16, kind="ExternalInput").ap()
    woT = nc.dram_tensor("woT", (NHL * HD, DIM), BF16, kind="ExternalInput").ap()
    # cosX rows 0-63 and 64-127 both hold cos; sinX rows 0-63 hold -sin,
    # rows 64-127 hold +sin (see host prep) -- lets RoPE run as 3 full-width
    # DVE ops with matching base partitions.  bf16 (rotation error ~0.2%)
    # to free SBUF for the softmax-sum accumulators.
    cosT = nc.dram_tensor("cosT", (128, S), BF16, kind="ExternalInput").ap()
    sinT = nc.dram_tensor("sinT", (128, S), BF16, kind="ExternalInput").ap()
    onesW = nc.dram_tensor("onesW", (P, P), BF16, kind="ExternalInput").ap()
    maskT = nc.dram_tensor("maskT", (P, P), F32, kind="ExternalInput").ap()
    out = nc.dram_tensor("out", (S, DIM), BF16, kind="ExternalOutput").ap()

    with tile.TileContext(nc) as tc, ExitStack() as ctx:
        const = ctx.enter_context(tc.tile_pool(name="const", bufs=1))
        resid = ctx.enter_context(tc.tile_pool(name="resid", bufs=1))
        xpool = ctx.enter_context(tc.tile_pool(name="xp", bufs=10))
        ptpool = ctx.enter_context(tc.tile_pool(name="ptp", bufs=12))
        tmppool = ctx.enter_context(tc.tile_pool(name="tmp", bufs=4))
        obpool = ctx.enter_context(tc.tile_pool(name="obp", bufs=5))
        psum = ctx.enter_context(tc.tile_pool(name="psum", bufs=8, space="PSUM"))

        # ---- startup streams: A0's diet (x chunk 0 + wqk + wv, 8.2MB) is
        # HBM-bound against A0's ~21us of PE work, so it is split across the
        # two DMA queues in the PE's k-group consumption order: even wqk
        # groups + all wv on the SWDGE queue, odd wqk groups (but g7)
        # interleaved with the x groups on the HWDGE queue. ----
        NKG = NKT // 4
        wqk_g = [const.tile([P, 4, QKM * P], BF16, tag=f"wqkg{g}", name=f"wqkg{g}")
                 for g in range(NKG)]
        wv_g = [const.tile([P, 4, HD], BF16, tag=f"wvg{g}", name=f"wvg{g}")
                for g in range(NKG)]

        def dma_wqk(g, eng):
            if g == 0:
                # per-k-tile DMAs for the first group: the first matmul can
                # start after one 160KB k-tile instead of the 640KB group.
                for j in range(4):
                    eng.dma_start(wqk_g[0][:, j],
                                  wqkT[j * P:(j + 1) * P, :])
                return
            eng.dma_start(
                wqk_g[g][:],
                wqkT[g * 4 * P:(g + 1) * 4 * P, :].rearrange(
                    "(kt p) m -> p kt m", p=P))

        def dma_wv(g, eng):
            eng.dma_start(
                wv_g[g][:],
                wvT[g * 4 * P:(g + 1) * 4 * P, :].rearrange(
                    "(kt p) m -> p kt m", p=P))

        x_ch = {}

        def load_x_chunk(ch, interleave=()):
            xgs = []
            for g in range(NKT // 4):
                xg = xpool.tile([P, 4, CHUNK], BF16, tag="xt", name="xt")
                nc.sync.dma_start(xg[:], xG[:, ch, g])
                xgs.append(xg)
                for wg in interleave:
                    if wg[0] == g:
                        dma_wqk(wg[1], nc.sync)
            x_ch[ch] = [xgs[k // 4][:, k % 4] for k in range(NKT)]

        for g in range(NKG):
            dma_wqk(g, nc.gpsimd)
            dma_wv(g, nc.gpsimd)
        wqk_sb = [wqk_g[k // 4][:, k % 4] for k in range(NKT)]
        wv_sb = [wv_g[k // 4][:, k % 4] for k in range(NKT)]
        mask_sb = const.tile([P, P], F32, tag="mask", name="mask")
        nc.gpsimd.dma_start(mask_sb[:], maskT[:])
        ones_sb = const.tile([P, P], BF16, tag="ones", name="ones")
        nc.gpsimd.dma_start(ones_sb[:], onesW[:])
        negC = const.tile([P, 1], F32, tag="negC", name="negC")
        nc.any.memset(negC[:], -C)
        wo_sb = const.tile([P, NHL, DIM], BF16, tag="wo", name="wo")
        nc.gpsimd.dma_start(wo_sb[:], woT.rearrange("(h p) n -> p h n", p=P))

        load_x_chunk(0)
        cos_sb = const.tile([P, S], BF16, tag="cos", name="cos")
        nc.sync.dma_start(cos_sb[:], cosT[:])
        sin_sb = const.tile([P, S], BF16, tag="sin", name="sin")
        nc.sync.dma_start(sin_sb[:], sinT[:])

        # resident activations (per chunk tiles for fine-grained deps)
        q_sb = [[resid.tile([P, CHUNK], BF16, tag=f"q{h}_{ch}", name=f"q{h}_{ch}")
                 for ch in range(NCH)] for h in range(NHL)]
        k_sb = [resid.tile([P, CHUNK], BF16, tag=f"k{ch}", name=f"k{ch}")
                for ch in range(NCH)]
        v_sb = [resid.tile([P, CHUNK], BF16, tag=f"v{ch}", name=f"v{ch}")
                for ch in range(NCH)]
        ot_sb = [[resid.tile([P, CHUNK], BF16, tag=f"ot{h}_{ch}", name=f"ot{h}_{ch}")
                  for ch in range(NCH)] for h in range(NHL)]

        def rope_hc(ps, raw_sw, out_tile, hc):
            """ps: (128, CHUNK//2) f32 PSUM [re; im]; raw_sw: bf16 SBUF with
            halves swapped [im; re] (produced by two ACT copies).
            out = ps*cosX + raw_sw*sinX with cosX = [cos; cos],
            sinX = [-sin; +sin]:
              out[0:64]   = re*cos - im*sin
              out[64:128] = im*cos + re*sin
            The cos mul reads PSUM directly (mixed PSUM+SBUF operands are
            exempt from the matching-base-partition rule); the ACT swap copies
            plus one DVE mul free the bank quickly while the remaining DVE ops
            run off-PSUM."""
            HC2 = CHUNK // 2
            cos = cos_sb[:, hc * HC2:(hc + 1) * HC2]
            sin = sin_sb[:, hc * HC2:(hc + 1) * HC2]
            t1 = tmppool.tile([P, HC2], F32, tag="t1", name="t1", bufs=2)
            t2 = tmppool.tile([P, HC2], F32, tag="t2", name="t2", bufs=2)
            nc.vector.tensor_mul(t1[:], ps[:], cos)
            nc.vector.tensor_mul(t2[:], raw_sw[:], sin)
            nc.vector.tensor_add(out_tile[:], t1[:], t2[:])

        HC = CHUNK // 2      # 256-wide half chunks: the qk PSUM footprint
        # drops to 3 banks (two heads packed per bank) + 1 shared V bank, so
        # phases B/C always find free banks and never stall on A's epilogue.
        vbank = [None]
        a_state = {}

        def phase_a(hc, skip_ep=False):
            """qkv projection (+ RoPE epilogue unless skip_ep) for half-chunk
            hc.  skip_ep defers the ACT/DVE epilogue emission (phase_a_ep) so
            another phase's exps can jump ahead of it in the ACT queue."""
            ch, half = hc // 2, hc % 2
            qk_bank = [psum.tile([P, CHUNK], F32, tag="ps", name="ps")
                       for _ in range((QKM + 1) // 2)]
            if half == 0:
                vbank[0] = psum.tile([P, CHUNK], F32, tag="ps", name="ps")
                if ch not in x_ch:
                    load_x_chunk(ch)
            ps_v = vbank[0]
            a_state[hc] = (qk_bank, ps_v)

            def qk_slice(m):
                return qk_bank[m // 2][:, (m % 2) * HC:(m % 2 + 1) * HC]

            for k in range(NKT):
                xt = x_ch[ch][k]
                xh = xt[:, half * HC:(half + 1) * HC]
                for m in range(QKM):
                    nc.tensor.matmul(
                        qk_slice(m), wqk_sb[k][:, m * P:(m + 1) * P], xh,
                        start=(k == 0 and m % 2 == 0),
                        stop=(k == NKT - 1 and (m % 2 == 1 or m == QKM - 1)),
                        skip_group_check=True)
                for t in range(2):
                    tt = 2 * half + t
                    nc.tensor.matmul(
                        ps_v[:, tt * P:(tt + 1) * P],
                        xt[:, tt * P:(tt + 1) * P], wv_sb[k][:],
                        start=(half == 0 and k == 0 and t == 0),
                        stop=(half == 1 and k == NKT - 1 and t == 1),
                        skip_group_check=True)
            if half == 1:
                del x_ch[ch]
            if not skip_ep:
                phase_a_ep(hc)

        def phase_a_ep(hc):
            """v copy + RoPE swap copies (ACT) and RoPE muls (DVE) for hc."""
            ch, half = hc // 2, hc % 2
            qk_bank, ps_v = a_state.pop(hc)

            def qk_slice(m):
                return qk_bank[m // 2][:, (m % 2) * HC:(m % 2 + 1) * HC]

            if half == 1:
                nc.scalar.copy(v_sb[ch][:], ps_v[:])
            rawsw = [tmppool.tile([P, HC], BF16, tag=f"qksw{m}", name=f"qksw{m}", bufs=2)
                     for m in range(QKM)]
            order = list(range(NHL)) + [NHL]     # banks free in alloc order
            for m in order:
                nc.scalar.copy(rawsw[m][0:64, :], qk_slice(m)[64:128, :])
                nc.scalar.copy(rawsw[m][64:128, :], qk_slice(m)[0:64, :])
            for m in order:
                out_tile = k_sb[ch] if m == NHL else q_sb[m][ch]
                rope_hc(qk_slice(m), rawsw[m],
                        out_tile[:, half * HC:(half + 1) * HC], hc)

        def score(ch, h, j):
            """score matmul + mask + exp for k-tile j of (head h, chunk ch)."""
            o = j - 4 * ch          # >=0: diagonal region, trim N
            lo = max(o, 0) * P      # first valid q column
            ps_st = psum.tile([P, CHUNK], F32, tag="ps", name="ps")
            nc.tensor.matmul(
                ps_st[:, lo:], k_sb[j // 4][:, (j % 4) * P:(j % 4 + 1) * P],
                q_sb[h][ch][:, lo:], start=True, stop=True)
            pt = ptpool.tile([P, CHUNK], BF16, tag="pt", name="pt")
            if o >= 0:  # mask the diagonal 128x128 block
                nc.vector.tensor_add(
                    ps_st[:, o * P:(o + 1) * P],
                    ps_st[:, o * P:(o + 1) * P], mask_sb[:])
            nc.scalar.activation(
                pt[:, lo:], ps_st[:, lo:],
                mybir.ActivationFunctionType.Exp, bias=negC[:])
            return pt, lo

        prescored = {}

        def finalize(ch, h, acc, ps_ot):
            """softmax denominators from the DVE-accumulated partial sums:
            one bf16 downcast + one 512-col ones-matmul per (head, chunk)
            replaces the per-k-tile ones-matmuls (saves ~25us of PE rows)."""
            acc_bf = tmppool.tile([P, CHUNK], BF16, tag="accbf", name="accbf",
                                  bufs=2)
            nc.vector.tensor_copy(out=acc_bf[:], in_=acc[:])
            ps_sum = psum.tile([P, CHUNK], F32, tag="ps", name="ps")
            nc.tensor.matmul(ps_sum[:], ones_sb[:], acc_bf[:],
                             start=True, stop=True)
            recip = tmppool.tile([P, CHUNK], F32, tag="recip", name="recip",
                                 bufs=2)
            nc.vector.reciprocal_approx_fast(out=recip[:], in_=ps_sum[:])
            nc.vector.tensor_mul(ot_sb[h][ch][:], ps_ot[:], recip[:])

        def phase_b(ch):
            """attention for all local heads, q chunk ch (causal).
            Software-pipelined: scores run 2 ahead of the exp-dependent PV
            matmuls.  Heads 0..2 accumulate pt into acc on the DVE (partition
            sums via one tiny matmul in finalize, emitted one head late so
            the DVE chain can drain under the next head's PE work); the last
            head keeps the classic per-k-tile ones-matmul so nothing ever
            defers past the end of the phase."""
            njt = 4 * ch + 4
            fin = None
            for h in range(NHL):
                classic = h == NHL - 1
                ps_ot = psum.tile([P, CHUNK], F32, tag="ps", name="ps")
                if classic:
                    ps_sum = psum.tile([P, CHUNK], F32, tag="ps", name="ps")
                else:
                    acc = tmppool.tile([P, CHUNK], F32, tag="acc", name="acc",
                                       bufs=2)
                if h == 0 and ch in prescored:
                    nxt = prescored.pop(ch)
                else:
                    nxt = [score(ch, h, 0), score(ch, h, 1)]
                for j in range(njt):
                    pt, lo = nxt.pop(0)
                    if j + 2 < njt:
                        nxt.append(score(ch, h, j + 2))
                    if classic:
                        nc.tensor.matmul(ps_sum[:, lo:], ones_sb[:],
                                         pt[:, lo:], start=(j == 0),
                                         stop=(j == njt - 1))
                    elif j == 0:
                        nc.vector.tensor_copy(out=acc[:], in_=pt[:])
                    else:
                        nc.vector.tensor_add(acc[:, lo:], acc[:, lo:],
                                             pt[:, lo:])
                    nc.tensor.matmul(
                        ps_ot[:, lo:], v_sb[j // 4][:, (j % 4) * P:(j % 4 + 1) * P],
                        pt[:, lo:], start=(j == 0), stop=(j == njt - 1))
                if fin is not None:
                    finalize(*fin)
                    fin = None
                if classic:
                    recip = tmppool.tile([P, CHUNK], F32, tag="recip",
                                         name="recip", bufs=2)
                    nc.vector.reciprocal_approx_fast(out=recip[:],
                                                     in_=ps_sum[:])
                    nc.vector.tensor_mul(ot_sb[h][ch][:], ps_ot[:], recip[:])
                else:
                    fin = (ch, h, acc, ps_ot)

        def phase_c(ch, pre_ch=None):
            """output projection for the 4 seq tiles of chunk ch.  The
            PSUM->SBUF bf16 copies alternate DVE/ACT so neither engine queues
            more than half of them ahead of the next B phase's exps.  pre_ch:
            emit the next B phase's first two scores+exps before the last
            quarter, hiding the exp latency at the B start."""
            for tq in range(4):
                if tq == 3 and pre_ch is not None:
                    prescored[pre_ch] = [score(pre_ch, 0, 0),
                                         score(pre_ch, 0, 1)]
                t = 4 * ch + tq
                for d in range(NDC):
                    ps_o = psum.tile([P, CHUNK], F32, tag="ps", name="ps")
                    for h in range(NHL):
                        nc.tensor.matmul(
                            ps_o[:], ot_sb[h][ch][:, tq * P:(tq + 1) * P],
                            wo_sb[:, h, d * CHUNK:(d + 1) * CHUNK],
                            start=(h == 0), stop=(h == NHL - 1))
                    ob = obpool.tile([P, CHUNK], BF16, tag="ob", name="ob")
                    if d % 2 == 0:
                        nc.vector.tensor_copy(out=ob[:], in_=ps_o[:])
                    else:
                        nc.scalar.copy(ob[:], ps_o[:])
                    oeng = nc.sync if d % 2 == 0 else nc.gpsimd
                    oeng.dma_start(
                        out[t * P:(t + 1) * P, d * CHUNK:(d + 1) * CHUNK], ob[:])

        # [A A C B] windows: C's PE work separates the A pair's ACT swap
        # copies (and C's own alternating copies) from B's exp stream, so by
        # the time the PE reaches B's matmuls ACT is drained and exps issue
        # back-to-back.  B0 (the one B with no C before it) eats a one-time
        # ~4us exp wait behind A3's swap copies; deferring those copies past
        # B0 would deadlock the shared 8-bank PSUM ring, so it stays.
        phase_a(0)
        phase_a(1)
        phase_a(2)
        phase_a(3)
        phase_b(0)
        for ch in range(2, NCH):
            phase_a(2 * ch)
            phase_a(2 * ch + 1)
            phase_c(ch - 2, pre_ch=ch - 1)
            phase_b(ch - 1)
        phase_c(NCH - 2, pre_ch=NCH - 1)
        phase_b(NCH - 1)
        phase_c(NCH - 1)

    return nc


def _make_in_maps(x, freqs_cis, wqkv, wo):
    scale = np.float32(1.0 / np.sqrt(HD))
    # chunk-major x: xG[p, ch, g, j, s] = x[0, ch*512+s, (4g+j)*128+p], so a
    # (ch, g) slice is one contiguous 4KB run per partition.
    xG = np.ascontiguousarray(
        np.asarray(x)[0].reshape(S // 512, 512, DIM // 512, 4, 128)
        .transpose(4, 0, 2, 3, 1)).astype(NPBF16)
    cos = freqs_cis[:, :, 0].T.astype(np.float32)        # (64, S)
    sin = freqs_cis[:, :, 1].T.astype(np.float32)
    cosT = np.ascontiguousarray(
        np.concatenate([cos, cos], 0)).astype(NPBF16)   # (128, S)
    sinT = np.ascontiguousarray(
        np.concatenate([-sin, sin], 0)).astype(NPBF16)
    ones = np.ones((128, 128), NPBF16)
    kp = np.arange(128)[:, None]
    qp = np.arange(128)[None, :]
    maskT = np.where(kp <= qp, 0.0, -1e30).astype(np.float32)

    in_maps = []
    for c in range(N_CORES):
        rows = [wqkv[128 * (NHL * c + h) + PERM] * scale for h in range(NHL)]
        rows.append(wqkv[NH * HD + 128 * c + PERM])
        wqkT = np.ascontiguousarray(np.concatenate(rows, 0).T).astype(NPBF16)
        wvT = np.ascontiguousarray(
            wqkv[(NH + NKV) * HD + 128 * c:(NH + NKV) * HD + 128 * (c + 1)].T
        ).astype(NPBF16)
        woT = np.ascontiguousarray(
            wo[:, 128 * NHL * c:128 * NHL * (c + 1)].T).astype(NPBF16)
        in_maps.append({
            "xG": xG, "wqkT": wqkT, "wvT": wvT, "woT": woT,
            "cosT": cosT, "sinT": sinT, "onesW": ones, "maskT": maskT,
        })
    return in_maps


def kernel(x, freqs_cis, wqkv, wo):
    x = np.asarray(x, dtype=np.float32)
    freqs_cis = np.asarray(freqs_cis, dtype=np.float32)
    wqkv = np.asarray(wqkv, dtype=np.float32)
    wo = np.asarray(wo, dtype=np.float32)

    in_maps = _make_in_maps(x, freqs_cis, wqkv, wo)
    nc = bacc.Bacc("TRN2", target_bir_lowering=False, debug=False,
                   num_devices=N_CORES)
    build_attention_kernel(nc, S=S, DIM=DIM)
    nc.compile()
    res = run_bass_kernel_spmd(nc, in_maps, core_ids=list(range(N_CORES)))

    acc = np.zeros((S, DIM), np.float32)
    for r in res.results:
        acc += np.asarray(r["out"]).astype(np.float32)
    return acc[None]


# revision 41
# speedup vs baseline: 1.2354x; 1.0469x over previous
"""Tensor-parallel GQA attention block on 8 TRN2 NeuronCores (Bass/Tile).

Problem: B=1, S=2048, DIM=4096, 32 q heads / 8 kv heads (GQA), head_dim=128,
RoPE, causal softmax, output projection.

Sharding (tensor parallel by head, per the hint): core c of 8 owns q heads
4c..4c+3 and kv head c (GQA groups stay with their q heads). wqkv rows and wo
columns are sharded by head; attention is fully local per core; each core
emits a partial (S, DIM) output (its heads through its wo column slice) and
the partials are summed on the host at unshard time (the "all-reduce after
wo" of the hint, done off-device since full I/O passes through the host
anyway).

Per-core device kernel -- all operands host-pre-transposed so every matmul has
its contraction dim on SBUF partitions; zero on-device transposes:
  qkT = wqkT.T @ xT              (head dims on partitions, seq free)
  v   = xT.T @ wvT               (seq on partitions, head dim free)
  RoPE on qT/kT in transposed layout: host permutes rows into re(0..63)/
    im(64..127); cos/sin arrive as stacked (128, S) tables [cos;cos] and
    [-sin;sin]; 1/sqrt(HD) is folded into wq on the host.
  per head, per 512-wide q chunk (causal: only k tiles <= chunk end):
    S.T[j] = kT_j.T @ qT_chunk   (k positions on partitions => softmax
                                  denominators via a ones-matmul; no P
                                  transpose anywhere)
    P.T[j] = exp(S.T[j] - 12)    (triangular mask added on diagonal tiles;
                                  N trimmed to the causal columns)
    sums  += ones128.T @ P.T[j]  (PSUM-accumulated, rows replicated)
    O.T   += matmul(lhsT=V_j, rhs=P.T[j])
    O.T_norm = O.T * reciprocal_approx(sums)  -> bf16
  out[t, d] = sum_h O.T_h[:, t].T @ woT_h[:, d]

Compute in bf16 with f32 PSUM accumulation; rel l2 error vs the f32 reference
is ~8e-3.  Performance structure: phases are emitted in [A A C B] windows so
the attention exp stream (the PE's only tight cross-engine dependency) is
never queued on ACT behind bulk copy work: output-projection PSUM->SBUF
copies alternate DVE/ACT and sit in the C slot where ACT is otherwise idle,
RoPE swap copies run on ACT during the A slots, exps run alone during B.
x streams in 512-wide tiles shared by the two half-chunk A phases; weights
stream in 4-k-tile groups on the SWDGE queue while x uses the HWDGE queue.
"""
import sys

sys.path.insert(0, "/opt/trn_rl_repo")

from contextlib import ExitStack

import numpy as np
import ml_dtypes

import concourse.bass as bass
import concourse.tile as tile
import concourse.mybir as mybir
from concourse import bacc
from concourse.bass_utils import run_bass_kernel_spmd

F32 = mybir.dt.float32
BF16 = mybir.dt.bfloat16
NPBF16 = ml_dtypes.bfloat16

NH, NKV, HD = 32, 8, 128
S, DIM = 2048, 4096
N_CORES = 8
NHL = NH // N_CORES          # q heads per core
PERM = np.concatenate([np.arange(0, 128, 2), np.arange(1, 128, 2)])


def build_attention_kernel(nc, S=2048, DIM=4096, C=12.0):
    NHL = 4          # local q heads
    HD = 128
    CHUNK = 512
    P = 128
    NKT = DIM // P         # k tiles over model dim
    NCH = S // CHUNK       # seq chunks
    QKM = NHL + 1          # m-tiles in qk GEMM (4 q heads + 1 k head)
    NDC = DIM // CHUNK     # output dim chunks

    # ---- DRAM I/O ----
    # x arrives chunk-major from the host: xG[p, ch, g, j, s] = x[ch*512+s,
    # (4g+j)*128+p], so one DMA pulls 4 k-tiles of one seq chunk as a single
    # contiguous 4KB run per partition (one descriptor per partition).
    xG = nc.dram_tensor("xG", (128, S // 512, DIM // 512, 4, 512), BF16,
                        kind="ExternalInput").ap()
    wqkT = nc.dram_tensor("wqkT", (DIM, QKM * P), BF16, kind="ExternalInput").ap()
    wvT = nc.dram_tensor("wvT", (DIM, HD), BF16, kind="ExternalInput").ap()
    woT = nc.dram_tensor("woT", (NHL * HD, DIM), BF16, kind="ExternalInput").ap()
    # cosX rows 0-63 and 64-127 both hold cos; sinX rows 0-63 hold -sin,
    # rows 64-127 hold +sin (see host prep) -- lets RoPE run as 3 full-width
    # DVE ops with matching base partitions.  bf16 (rotation error ~0.2%)
    # to free SBUF for the softmax-sum accumulators.
    cosT = nc.dram_tensor("cosT", (128, S), BF16, kind="ExternalInput").ap()
    sinT = nc.dram_tensor("sinT", (128, S), BF16, kind="ExternalInput").ap()
    onesW = nc.dram_tensor("onesW", (P, P), BF16, kind="ExternalInput").ap()
    maskT = nc.dram_tensor("maskT", (P, P), F32, kind="ExternalInput").ap()
    out = nc.dram_tensor("out", (S, DIM), BF16, kind="ExternalOutput").ap()

    with tile.TileContext(nc) as tc, ExitStack() as ctx:
        const = ctx.enter_context(tc.tile_pool(name="const", bufs=1))
        resid = ctx.enter_context(tc.tile_pool(name="resid", bufs=1))
        xpool = ctx.enter_context(tc.tile_pool(name="xp", bufs=10))
        ptpool = ctx.enter_context(tc.tile_pool(name="ptp", bufs=12))
        tmppool = ctx.enter_context(tc.tile_pool(name="tmp", bufs=4))
        obpool = ctx.enter_context(tc.tile_pool(name="obp", bufs=8))
        psum = ctx.enter_context(tc.tile_pool(name="psum", bufs=8, space="PSUM"))

        # ---- startup streams: A0's diet (x chunk 0 + wqk + wv, 8.2MB) is
        # HBM-bound against A0's ~21us of PE work, so it is split across the
        # two DMA queues in the PE's k-group consumption order: even wqk
        # groups + all wv on the SWDGE queue, odd wqk groups (but g7)
        # interleaved with the x groups on the HWDGE queue. ----
        NKG = NKT // 4
        wqk_g = [const.tile([P, 4, QKM * P], BF16, tag=f"wqkg{g}", name=f"wqkg{g}")
                 for g in range(NKG)]
        wv_g = [const.tile([P, 4, HD], BF16, tag=f"wvg{g}", name=f"wvg{g}")
                for g in range(NKG)]

        def dma_wqk(g, eng):
            if g == 0:
                # per-k-tile DMAs for the first group: the first matmul can
                # start after one 160KB k-tile instead of the 640KB group.
                for j in range(4):
                    eng.dma_start(wqk_g[0][:, j],
                                  wqkT[j * P:(j + 1) * P, :])
                return
            eng.dma_start(
                wqk_g[g][:],
                wqkT[g * 4 * P:(g + 1) * 4 * P, :].rearrange(
                    "(kt p) m -> p kt m", p=P))

        def dma_wv(g, eng):
            eng.dma_start(
                wv_g[g][:],
                wvT[g * 4 * P:(g + 1) * 4 * P, :].rearrange(
                    "(kt p) m -> p kt m", p=P))

        x_ch = {}

        def load_x_chunk(ch, interleave=()):
            xgs = []
            for g in range(NKT // 4):
                xg = xpool.tile([P, 4, CHUNK], BF16, tag="xt", name="xt")
                nc.sync.dma_start(xg[:], xG[:, ch, g])
                xgs.append(xg)
                for wg in interleave:
                    if wg[0] == g:
                        dma_wqk(wg[1], nc.sync)
            x_ch[ch] = [xgs[k // 4][:, k % 4] for k in range(NKT)]

        for g in range(NKG):
            dma_wqk(g, nc.gpsimd)
            dma_wv(g, nc.gpsimd)
        wqk_sb = [wqk_g[k // 4][:, k % 4] for k in range(NKT)]
        wv_sb = [wv_g[k // 4][:, k % 4] for k in range(NKT)]
        mask_sb = const.tile([P, P], F32, tag="mask", name="mask")
        nc.gpsimd.dma_start(mask_sb[:], maskT[:])
        ones_sb = const.tile([P, P], BF16, tag="ones", name="ones")
        nc.gpsimd.dma_start(ones_sb[:], onesW[:])
        negC = const.tile([P, 1], F32, tag="negC", name="negC")
        nc.any.memset(negC[:], -C)
        wo_sb = const.tile([P, NHL, DIM], BF16, tag="wo", name="wo")
        nc.gpsimd.dma_start(wo_sb[:], woT.rearrange("(h p) n -> p h n", p=P))

        load_x_chunk(0)
        cos_sb = const.tile([P, S], BF16, tag="cos", name="cos")
        nc.sync.dma_start(cos_sb[:], cosT[:])
        sin_sb = const.tile([P, S], BF16, tag="sin", name="sin")
        nc.sync.dma_start(sin_sb[:], sinT[:])

        # resident activations (per chunk tiles for fine-grained deps)
        q_sb = [[resid.tile([P, CHUNK], BF16, tag=f"q{h}_{ch}", name=f"q{h}_{ch}")
                 for ch in range(NCH)] for h in range(NHL)]
        k_sb = [resid.tile([P, CHUNK], BF16, tag=f"k{ch}", name=f"k{ch}")
                for ch in range(NCH)]
        v_sb = [resid.tile([P, CHUNK], BF16, tag=f"v{ch}", name=f"v{ch}")
                for ch in range(NCH)]
        ot_sb = [[resid.tile([P, CHUNK], BF16, tag=f"ot{h}_{ch}", name=f"ot{h}_{ch}")
                  for ch in range(NCH)] for h in range(NHL)]

        def rope_hc(ps, raw_sw, out_tile, hc):
            """ps: (128, CHUNK//2) f32 PSUM [re; im]; raw_sw: bf16 SBUF with
            halves swapped [im; re] (produced by two ACT copies).
            out = ps*cosX + raw_sw*sinX with cosX = [cos; cos],
            sinX = [-sin; +sin]:
              out[0:64]   = re*cos - im*sin
              out[64:128] = im*cos + re*sin
            The cos mul reads PSUM directly (mixed PSUM+SBUF operands are
            exempt from the matching-base-partition rule); the ACT swap copies
            plus one DVE mul free the bank quickly while the remaining DVE ops
            run off-PSUM."""
            HC2 = CHUNK // 2
            cos = cos_sb[:, hc * HC2:(hc + 1) * HC2]
            sin = sin_sb[:, hc * HC2:(hc + 1) * HC2]
            t1 = tmppool.tile([P, HC2], F32, tag="t1", name="t1", bufs=2)
            t2 = tmppool.tile([P, HC2], F32, tag="t2", name="t2", bufs=2)
            nc.vector.tensor_mul(t1[:], ps[:], cos)
            nc.vector.tensor_mul(t2[:], raw_sw[:], sin)
            nc.vector.tensor_add(out_tile[:], t1[:], t2[:])

        HC = CHUNK // 2      # 256-wide half chunks: the qk PSUM footprint
        # drops to 3 banks (two heads packed per bank) + 1 shared V bank, so
        # phases B/C always find free banks and never stall on A's epilogue.
        vbank = [None]
        a_state = {}

        def phase_a(hc, skip_ep=False, mid_cb=None):
            """qkv projection (+ RoPE epilogue unless skip_ep) for half-chunk
            hc.  mid_cb is invoked two-thirds into the k loop: work emitted
            there runs on the idle ACT/DVE engines under this phase's
            matmuls, ahead of the epilogue's swap copies."""
            ch, half = hc // 2, hc % 2
            qk_bank = [psum.tile([P, CHUNK], F32, tag="ps", name="ps")
                       for _ in range((QKM + 1) // 2)]
            if half == 0:
                vbank[0] = psum.tile([P, CHUNK], F32, tag="ps", name="ps")
                if ch not in x_ch:
                    load_x_chunk(ch)
            ps_v = vbank[0]
            a_state[hc] = (qk_bank, ps_v)

            def qk_slice(m):
                return qk_bank[m // 2][:, (m % 2) * HC:(m % 2 + 1) * HC]

            for k in range(NKT):
                if k == 20 and mid_cb is not None:
                    mid_cb()
                xt = x_ch[ch][k]
                xh = xt[:, half * HC:(half + 1) * HC]
                for m in range(QKM):
                    nc.tensor.matmul(
                        qk_slice(m), wqk_sb[k][:, m * P:(m + 1) * P], xh,
                        start=(k == 0 and m % 2 == 0),
                        stop=(k == NKT - 1 and (m % 2 == 1 or m == QKM - 1)),
                        skip_group_check=True)
                for t in range(2):
                    tt = 2 * half + t
                    nc.tensor.matmul(
                        ps_v[:, tt * P:(tt + 1) * P],
                        xt[:, tt * P:(tt + 1) * P], wv_sb[k][:],
                        start=(half == 0 and k == 0 and t == 0),
                        stop=(half == 1 and k == NKT - 1 and t == 1),
                        skip_group_check=True)
            if half == 1:
                del x_ch[ch]
            if not skip_ep:
                phase_a_ep(hc)

        def phase_a_ep(hc):
            """v copy + RoPE swap copies (ACT) and RoPE muls (DVE) for hc."""
            ch, half = hc // 2, hc % 2
            qk_bank, ps_v = a_state.pop(hc)

            def qk_slice(m):
                return qk_bank[m // 2][:, (m % 2) * HC:(m % 2 + 1) * HC]

            if half == 1:
                nc.scalar.copy(v_sb[ch][:], ps_v[:])
            rawsw = [tmppool.tile([P, HC], BF16, tag=f"qksw{m}", name=f"qksw{m}", bufs=2)
                     for m in range(QKM)]
            order = list(range(NHL)) + [NHL]     # banks free in alloc order
            for m in order:
                nc.scalar.copy(rawsw[m][0:64, :], qk_slice(m)[64:128, :])
                nc.scalar.copy(rawsw[m][64:128, :], qk_slice(m)[0:64, :])
            for m in order:
                out_tile = k_sb[ch] if m == NHL else q_sb[m][ch]
                rope_hc(qk_slice(m), rawsw[m],
                        out_tile[:, half * HC:(half + 1) * HC], hc)

        def score(ch, h, j):
            """score matmul + mask + exp for k-tile j of (head h, chunk ch)."""
            o = j - 4 * ch          # >=0: diagonal region, trim N
            lo = max(o, 0) * P      # first valid q column
            ps_st = psum.tile([P, CHUNK], F32, tag="ps", name="ps")
            nc.tensor.matmul(
                ps_st[:, lo:], k_sb[j // 4][:, (j % 4) * P:(j % 4 + 1) * P],
                q_sb[h][ch][:, lo:], start=True, stop=True)
            pt = ptpool.tile([P, CHUNK], BF16, tag="pt", name="pt")
            if o >= 0:  # mask the diagonal 128x128 block
                nc.vector.tensor_add(
                    ps_st[:, o * P:(o + 1) * P],
                    ps_st[:, o * P:(o + 1) * P], mask_sb[:])
            nc.scalar.activation(
                pt[:, lo:], ps_st[:, lo:],
                mybir.ActivationFunctionType.Exp, bias=negC[:])
            return pt, lo

        prescored = {}

        def phase_b(ch):
            """attention for all local heads, q chunk ch (causal).
            Software-pipelined: scores run 2 ahead of the exp-dependent
            sums/PV matmuls, so the PE rarely waits on ACT.  prescored[ch]
            holds per-head score lists already emitted by an earlier phase."""
            njt = 4 * ch + 4
            pre = prescored.pop(ch, {})
            for h in range(NHL):
                ps_sum = psum.tile([P, CHUNK], F32, tag="ps", name="ps")
                ps_ot = psum.tile([P, CHUNK], F32, tag="ps", name="ps")
                nxt = pre.get(h, [])
                next_j = len(nxt)
                while len(nxt) < 2 and next_j < njt:
                    nxt.append(score(ch, h, next_j))
                    next_j += 1
                for j in range(njt):
                    pt, lo = nxt.pop(0)
                    if next_j < njt:
                        nxt.append(score(ch, h, next_j))
                        next_j += 1
                    nc.tensor.matmul(ps_sum[:, lo:], ones_sb[:], pt[:, lo:],
                                     start=(j == 0), stop=(j == njt - 1))
                    nc.tensor.matmul(
                        ps_ot[:, lo:], v_sb[j // 4][:, (j % 4) * P:(j % 4 + 1) * P],
                        pt[:, lo:], start=(j == 0), stop=(j == njt - 1))
                recip = tmppool.tile([P, CHUNK], F32, tag="recip", name="recip", bufs=2)
                nc.vector.reciprocal_approx_fast(out=recip[:], in_=ps_sum[:])
                nc.vector.tensor_mul(ot_sb[h][ch][:], ps_ot[:], recip[:])

        def phase_c(ch, pre_ch=None):
            """output projection for the 4 seq tiles of chunk ch.  The
            PSUM->SBUF bf16 copies alternate DVE/ACT so neither engine queues
            more than half of them ahead of the next B phase's exps.  pre_ch:
            emit the next B phase's first two scores+exps before the last
            quarter, hiding the exp latency at the B start."""
            for tq in range(4):
                if tq == 3 and pre_ch is not None:
                    prescored[pre_ch] = {0: [score(pre_ch, 0, 0),
                                             score(pre_ch, 0, 1)]}
                t = 4 * ch + tq
                for d in range(NDC):
                    ps_o = psum.tile([P, CHUNK], F32, tag="ps", name="ps")
                    for h in range(NHL):
                        nc.tensor.matmul(
                            ps_o[:], ot_sb[h][ch][:, tq * P:(tq + 1) * P],
                            wo_sb[:, h, d * CHUNK:(d + 1) * CHUNK],
                            start=(h == 0), stop=(h == NHL - 1))
                    ob = obpool.tile([P, CHUNK], BF16, tag="ob", name="ob")
                    if d % 2 == 0:
                        nc.vector.tensor_copy(out=ob[:], in_=ps_o[:])
                    else:
                        nc.scalar.copy(ob[:], ps_o[:])
                    oeng = nc.sync if d % 2 == 0 else nc.gpsimd
                    oeng.dma_start(
                        out[t * P:(t + 1) * P, d * CHUNK:(d + 1) * CHUNK], ob[:])

        # [A A C B] windows: C's PE work separates the A pair's ACT swap
        # copies (and C's own alternating copies) from B's exp stream, so by
        # the time the PE reaches B's matmuls ACT is drained and exps issue
        # back-to-back.  B0 (the one B with no C before it) would wait ~4.5us
        # for exps queued behind A3's swap copies, so most of its scores+exps
        # are emitted mid-A3 where ACT is idle (deferring A3's copies past B0
        # instead would deadlock the shared 8-bank PSUM ring).
        def pre_b0():
            prescored[0] = {
                0: [score(0, 0, j) for j in range(4)],
                1: [score(0, 1, j) for j in range(4)],
                2: [score(0, 2, j) for j in range(2)],
            }

        phase_a(0)
        phase_a(1)
        phase_a(2)
        phase_a(3, mid_cb=pre_b0)
        phase_b(0)
        for ch in range(2, NCH):
            phase_a(2 * ch)
            phase_a(2 * ch + 1)
            phase_c(ch - 2, pre_ch=ch - 1)
            phase_b(ch - 1)
        phase_c(NCH - 2, pre_ch=NCH - 1)
        phase_b(NCH - 1)
        phase_c(NCH - 1)

    return nc


def _make_in_maps(x, freqs_cis, wqkv, wo):
    scale = np.float32(1.0 / np.sqrt(HD))
    # chunk-major x: xG[p, ch, g, j, s] = x[0, ch*512+s, (4g+j)*128+p], so a
    # (ch, g) slice is one contiguous 4KB run per partition.
    xG = np.ascontiguousarray(
        np.asarray(x)[0].reshape(S // 512, 512, DIM // 512, 4, 128)
        .transpose(4, 0, 2, 3, 1)).astype(NPBF16)
    cos = freqs_cis[:, :, 0].T.astype(np.float32)        # (64, S)
    sin = freqs_cis[:, :, 1].T.astype(np.float32)
    cosT = np.ascontiguousarray(
        np.concatenate([cos, cos], 0)).astype(NPBF16)   # (128, S)
    sinT = np.ascontiguousarray(
        np.concatenate([-sin, sin], 0)).astype(NPBF16)
    ones = np.ones((128, 128), NPBF16)
    kp = np.arange(128)[:, None]
    qp = np.arange(128)[None, :]
    maskT = np.where(kp <= qp, 0.0, -1e30).astype(np.float32)

    in_maps = []
    for c in range(N_CORES):
        rows = [wqkv[128 * (NHL * c + h) + PERM] * scale for h in range(NHL)]
        rows.append(wqkv[NH * HD + 128 * c + PERM])
        wqkT = np.ascontiguousarray(np.concatenate(rows, 0).T).astype(NPBF16)
        wvT = np.ascontiguousarray(
            wqkv[(NH + NKV) * HD + 128 * c:(NH + NKV) * HD + 128 * (c + 1)].T
        ).astype(NPBF16)
        woT = np.ascontiguousarray(
            wo[:, 128 * NHL * c:128 * NHL * (c + 1)].T).astype(NPBF16)
        in_maps.append({
            "xG": xG, "wqkT": wqkT, "wvT": wvT, "woT": woT,
            "cosT": cosT, "sinT": sinT, "onesW": ones, "maskT": maskT,
        })
    return in_maps


def kernel(x, freqs_cis, wqkv, wo):
    x = np.asarray(x, dtype=np.float32)
    freqs_cis = np.asarray(freqs_cis, dtype=np.float32)
    wqkv = np.asarray(wqkv, dtype=np.float32)
    wo = np.asarray(wo, dtype=np.float32)

    in_maps = _make_in_maps(x, freqs_cis, wqkv, wo)
    nc = bacc.Bacc("TRN2", target_bir_lowering=False, debug=False,
                   num_devices=N_CORES)
    build_attention_kernel(nc, S=S, DIM=DIM)
    nc.compile()
    res = run_bass_kernel_spmd(nc, in_maps, core_ids=list(range(N_CORES)))

    acc = np.zeros((S, DIM), np.float32)
    for r in res.results:
        acc += np.asarray(r["out"]).astype(np.float32)
    return acc[None]


# revision 45
# speedup vs baseline: 1.2426x; 1.0058x over previous
"""Tensor-parallel GQA attention block on 8 TRN2 NeuronCores (Bass/Tile).

Problem: B=1, S=2048, DIM=4096, 32 q heads / 8 kv heads (GQA), head_dim=128,
RoPE, causal softmax, output projection.

Sharding (tensor parallel by head, per the hint): core c of 8 owns q heads
4c..4c+3 and kv head c (GQA groups stay with their q heads). wqkv rows and wo
columns are sharded by head; attention is fully local per core; each core
emits a partial (S, DIM) output (its heads through its wo column slice) and
the partials are summed on the host at unshard time (the "all-reduce after
wo" of the hint, done off-device since full I/O passes through the host
anyway).

Per-core device kernel -- all operands host-pre-transposed so every matmul has
its contraction dim on SBUF partitions; zero on-device transposes:
  qkT = wqkT.T @ xT              (head dims on partitions, seq free)
  v   = xT.T @ wvT               (seq on partitions, head dim free)
  RoPE on qT/kT in transposed layout: host permutes rows into re(0..63)/
    im(64..127); cos/sin arrive as stacked (128, S) tables [cos;cos] and
    [-sin;sin]; 1/sqrt(HD) is folded into wq on the host.
  per head, per 512-wide q chunk (causal: only k tiles <= chunk end):
    S.T[j] = kT_j.T @ qT_chunk   (k positions on partitions => softmax
                                  denominators via a ones-matmul; no P
                                  transpose anywhere)
    P.T[j] = exp(S.T[j] - 12)    (triangular mask added on diagonal tiles;
                                  N trimmed to the causal columns)
    sums  += ones128.T @ P.T[j]  (PSUM-accumulated, rows replicated)
    O.T   += matmul(lhsT=V_j, rhs=P.T[j])
    O.T_norm = O.T * reciprocal_approx(sums)  -> bf16
  out[t, d] = sum_h O.T_h[:, t].T @ woT_h[:, d]

Compute in bf16 with f32 PSUM accumulation; rel l2 error vs the f32 reference
is ~8e-3.  Performance structure: phases are emitted in [A A C B] windows so
the attention exp stream (the PE's only tight cross-engine dependency) is
never queued on ACT behind bulk copy work: output-projection PSUM->SBUF
copies alternate DVE/ACT and sit in the C slot where ACT is otherwise idle,
RoPE swap copies run on ACT during the A slots, exps run alone during B.
x streams in 512-wide tiles shared by the two half-chunk A phases; weights
stream in 4-k-tile groups on the SWDGE queue while x uses the HWDGE queue.
"""
import sys

sys.path.insert(0, "/opt/trn_rl_repo")

from contextlib import ExitStack

import numpy as np
import ml_dtypes

import concourse.bass as bass
import concourse.tile as tile
import concourse.mybir as mybir
from concourse import bacc
from concourse.bass_utils import run_bass_kernel_spmd

F32 = mybir.dt.float32
BF16 = mybir.dt.bfloat16
NPBF16 = ml_dtypes.bfloat16

NH, NKV, HD = 32, 8, 128
S, DIM = 2048, 4096
N_CORES = 8
NHL = NH // N_CORES          # q heads per core
PERM = np.concatenate([np.arange(0, 128, 2), np.arange(1, 128, 2)])


def build_attention_kernel(nc, S=2048, DIM=4096, C=12.0):
    NHL = 4          # local q heads
    HD = 128
    CHUNK = 512
    P = 128
    NKT = DIM // P         # k tiles over model dim
    NCH = S // CHUNK       # seq chunks
    QKM = NHL + 1          # m-tiles in qk GEMM (4 q heads + 1 k head)
    NDC = DIM // CHUNK     # output dim chunks

    # ---- DRAM I/O ----
    # x arrives chunk-major from the host: xG[p, ch, g, j, s] = x[ch*512+s,
    # (4g+j)*128+p], so one DMA pulls 4 k-tiles of one seq chunk as a single
    # contiguous 4KB run per partition (one descriptor per partition).
    xG = nc.dram_tensor("xG", (128, S // 512, DIM // 512, 4, 512), BF16,
                        kind="ExternalInput").ap()
    wqkT = nc.dram_tensor("wqkT", (DIM, QKM * P), BF16, kind="ExternalInput").ap()
    wvT = nc.dram_tensor("wvT", (DIM, HD), BF16, kind="ExternalInput").ap()
    woT = nc.dram_tensor("woT", (NHL * HD, DIM), BF16, kind="ExternalInput").ap()
    # cosX rows 0-63 and 64-127 both hold cos; sinX rows 0-63 hold -sin,
    # rows 64-127 hold +sin (see host prep) -- lets RoPE run as 3 full-width
    # DVE ops with matching base partitions.  bf16 (rotation error ~0.2%)
    # to free SBUF for the softmax-sum accumulators.
    cosT = nc.dram_tensor("cosT", (128, S), BF16, kind="ExternalInput").ap()
    sinT = nc.dram_tensor("sinT", (128, S), BF16, kind="ExternalInput").ap()
    onesW = nc.dram_tensor("onesW", (P, P), BF16, kind="ExternalInput").ap()
    maskT = nc.dram_tensor("maskT", (P, P), F32, kind="ExternalInput").ap()
    out = nc.dram_tensor("out", (S, DIM), BF16, kind="ExternalOutput").ap()

    with tile.TileContext(nc) as tc, ExitStack() as ctx:
        const = ctx.enter_context(tc.tile_pool(name="const", bufs=1))
        resid = ctx.enter_context(tc.tile_pool(name="resid", bufs=1))
        xpool = ctx.enter_context(tc.tile_pool(name="xp", bufs=10))
        ptpool = ctx.enter_context(tc.tile_pool(name="ptp", bufs=16))
        tmppool = ctx.enter_context(tc.tile_pool(name="tmp", bufs=4))
        obpool = ctx.enter_context(tc.tile_pool(name="obp", bufs=8))
        psum = ctx.enter_context(tc.tile_pool(name="psum", bufs=8, space="PSUM"))

        # ---- startup streams: A0's diet (x chunk 0 + wqk + wv, 8.2MB) is
        # HBM-bound against A0's ~21us of PE work, so it is split across the
        # two DMA queues in the PE's k-group consumption order: even wqk
        # groups + all wv on the SWDGE queue, odd wqk groups (but g7)
        # interleaved with the x groups on the HWDGE queue. ----
        NKG = NKT // 4
        wqk_g = [const.tile([P, 4, QKM * P], BF16, tag=f"wqkg{g}", name=f"wqkg{g}")
                 for g in range(NKG)]
        wv_g = [const.tile([P, 4, HD], BF16, tag=f"wvg{g}", name=f"wvg{g}")
                for g in range(NKG)]

        def dma_wqk(g, eng):
            if g == 0:
                # per-k-tile DMAs for the first group: the first matmul can
                # start after one 160KB k-tile instead of the 640KB group.
                for j in range(4):
                    eng.dma_start(wqk_g[0][:, j],
                                  wqkT[j * P:(j + 1) * P, :])
                return
            # 2-k-tile halves: arrival granularity ~1us against the PE's
            # ~0.64us/k-tile consumption, halving the per-group burst wait.
            for h in range(2):
                eng.dma_start(
                    wqk_g[g][:, 2 * h:2 * h + 2],
                    wqkT[(4 * g + 2 * h) * P:(4 * g + 2 * h + 2) * P,
                         :].rearrange("(kt p) m -> p kt m", p=P))

        def dma_wv(g, eng):
            eng.dma_start(
                wv_g[g][:],
                wvT[g * 4 * P:(g + 1) * 4 * P, :].rearrange(
                    "(kt p) m -> p kt m", p=P))

        x_ch = {}

        def load_x_chunk(ch, interleave=()):
            xgs = []
            for g in range(NKT // 4):
                xg = xpool.tile([P, 4, CHUNK], BF16, tag="xt", name="xt")
                nc.sync.dma_start(xg[:], xG[:, ch, g])
                xgs.append(xg)
                for wg in interleave:
                    if wg[0] == g:
                        dma_wqk(wg[1], nc.sync)
            x_ch[ch] = [xgs[k // 4][:, k % 4] for k in range(NKT)]

        for g in range(NKG):
            dma_wqk(g, nc.gpsimd)
            dma_wv(g, nc.gpsimd)
        wqk_sb = [wqk_g[k // 4][:, k % 4] for k in range(NKT)]
        wv_sb = [wv_g[k // 4][:, k % 4] for k in range(NKT)]
        mask_sb = const.tile([P, P], F32, tag="mask", name="mask")
        nc.gpsimd.dma_start(mask_sb[:], maskT[:])
        ones_sb = const.tile([P, P], BF16, tag="ones", name="ones")
        nc.gpsimd.dma_start(ones_sb[:], onesW[:])
        negC = const.tile([P, 1], F32, tag="negC", name="negC")
        nc.any.memset(negC[:], -C)
        wo_sb = const.tile([P, NHL, DIM], BF16, tag="wo", name="wo")
        nc.gpsimd.dma_start(wo_sb[:], woT.rearrange("(h p) n -> p h n", p=P))

        load_x_chunk(0)
        cos_sb = const.tile([P, S], BF16, tag="cos", name="cos")
        nc.sync.dma_start(cos_sb[:], cosT[:])
        sin_sb = const.tile([P, S], BF16, tag="sin", name="sin")
        nc.sync.dma_start(sin_sb[:], sinT[:])

        # resident activations (per chunk tiles for fine-grained deps)
        q_sb = [[resid.tile([P, CHUNK], BF16, tag=f"q{h}_{ch}", name=f"q{h}_{ch}")
                 for ch in range(NCH)] for h in range(NHL)]
        k_sb = [resid.tile([P, CHUNK], BF16, tag=f"k{ch}", name=f"k{ch}")
                for ch in range(NCH)]
        v_sb = [resid.tile([P, CHUNK], BF16, tag=f"v{ch}", name=f"v{ch}")
                for ch in range(NCH)]
        ot_sb = [[resid.tile([P, CHUNK], BF16, tag=f"ot{h}_{ch}", name=f"ot{h}_{ch}")
                  for ch in range(NCH)] for h in range(NHL)]

        def rope_hc(ps, raw_sw, out_tile, hc):
            """ps: (128, CHUNK//2) f32 PSUM [re; im]; raw_sw: bf16 SBUF with
            halves swapped [im; re] (produced by two ACT copies).
            out = ps*cosX + raw_sw*sinX with cosX = [cos; cos],
            sinX = [-sin; +sin]:
              out[0:64]   = re*cos - im*sin
              out[64:128] = im*cos + re*sin
            The cos mul reads PSUM directly (mixed PSUM+SBUF operands are
            exempt from the matching-base-partition rule); the ACT swap copies
            plus one DVE mul free the bank quickly while the remaining DVE ops
            run off-PSUM."""
            HC2 = CHUNK // 2
            cos = cos_sb[:, hc * HC2:(hc + 1) * HC2]
            sin = sin_sb[:, hc * HC2:(hc + 1) * HC2]
            t1 = tmppool.tile([P, HC2], F32, tag="t1", name="t1", bufs=2)
            t2 = tmppool.tile([P, HC2], F32, tag="t2", name="t2", bufs=2)
            nc.vector.tensor_mul(t1[:], ps[:], cos)
            nc.vector.tensor_mul(t2[:], raw_sw[:], sin)
            nc.vector.tensor_add(out_tile[:], t1[:], t2[:])

        HC = CHUNK // 2      # 256-wide half chunks: the qk PSUM footprint
        # drops to 3 banks (two heads packed per bank) + 1 shared V bank, so
        # phases B/C always find free banks and never stall on A's epilogue.
        vbank = [None]
        a_state = {}

        def phase_a(hc, skip_ep=False, mid_cb=None):
            """qkv projection (+ RoPE epilogue unless skip_ep) for half-chunk
            hc.  mid_cb is invoked two-thirds into the k loop: work emitted
            there runs on the idle ACT/DVE engines under this phase's
            matmuls, ahead of the epilogue's swap copies."""
            ch, half = hc // 2, hc % 2
            qk_bank = [psum.tile([P, CHUNK], F32, tag="ps", name="ps")
                       for _ in range((QKM + 1) // 2)]
            if half == 0:
                vbank[0] = psum.tile([P, CHUNK], F32, tag="ps", name="ps")
                if ch not in x_ch:
                    load_x_chunk(ch)
            ps_v = vbank[0]
            a_state[hc] = (qk_bank, ps_v)

            def qk_slice(m):
                return qk_bank[m // 2][:, (m % 2) * HC:(m % 2 + 1) * HC]

            for k in range(NKT):
                if k == 12 and mid_cb is not None:
                    mid_cb()
                xt = x_ch[ch][k]
                xh = xt[:, half * HC:(half + 1) * HC]
                for m in range(QKM):
                    nc.tensor.matmul(
                        qk_slice(m), wqk_sb[k][:, m * P:(m + 1) * P], xh,
                        start=(k == 0 and m % 2 == 0),
                        stop=(k == NKT - 1 and (m % 2 == 1 or m == QKM - 1)),
                        skip_group_check=True)
                for t in range(2):
                    tt = 2 * half + t
                    nc.tensor.matmul(
                        ps_v[:, tt * P:(tt + 1) * P],
                        xt[:, tt * P:(tt + 1) * P], wv_sb[k][:],
                        start=(half == 0 and k == 0 and t == 0),
                        stop=(half == 1 and k == NKT - 1 and t == 1),
                        skip_group_check=True)
            if half == 1:
                del x_ch[ch]
            if not skip_ep:
                phase_a_ep(hc)

        def phase_a_ep(hc):
            """v copy + RoPE swap copies (ACT) and RoPE muls (DVE) for hc."""
            ch, half = hc // 2, hc % 2
            qk_bank, ps_v = a_state.pop(hc)

            def qk_slice(m):
                return qk_bank[m // 2][:, (m % 2) * HC:(m % 2 + 1) * HC]

            if half == 1:
                nc.scalar.copy(v_sb[ch][:], ps_v[:])
            rawsw = [tmppool.tile([P, HC], BF16, tag=f"qksw{m}", name=f"qksw{m}", bufs=2)
                     for m in range(QKM)]
            order = list(range(NHL)) + [NHL]     # banks free in alloc order
            for m in order:
                nc.scalar.copy(rawsw[m][0:64, :], qk_slice(m)[64:128, :])
                nc.scalar.copy(rawsw[m][64:128, :], qk_slice(m)[0:64, :])
            for m in order:
                out_tile = k_sb[ch] if m == NHL else q_sb[m][ch]
                rope_hc(qk_slice(m), rawsw[m],
                        out_tile[:, half * HC:(half + 1) * HC], hc)

        def score(ch, h, j):
            """score matmul + mask + exp for k-tile j of (head h, chunk ch)."""
            o = j - 4 * ch          # >=0: diagonal region, trim N
            lo = max(o, 0) * P      # first valid q column
            ps_st = psum.tile([P, CHUNK], F32, tag="ps", name="ps")
            nc.tensor.matmul(
                ps_st[:, lo:], k_sb[j // 4][:, (j % 4) * P:(j % 4 + 1) * P],
                q_sb[h][ch][:, lo:], start=True, stop=True)
            pt = ptpool.tile([P, CHUNK], BF16, tag="pt", name="pt")
            if o >= 0:  # mask the diagonal 128x128 block
                nc.vector.tensor_add(
                    ps_st[:, o * P:(o + 1) * P],
                    ps_st[:, o * P:(o + 1) * P], mask_sb[:])
            nc.scalar.activation(
                pt[:, lo:], ps_st[:, lo:],
                mybir.ActivationFunctionType.Exp, bias=negC[:])
            return pt, lo

        prescored = {}

        def phase_b(ch):
            """attention for all local heads, q chunk ch (causal).
            Software-pipelined: scores run 2 ahead of the exp-dependent
            sums/PV matmuls, so the PE rarely waits on ACT.  prescored[ch]
            holds per-head score lists already emitted by an earlier phase."""
            njt = 4 * ch + 4
            pre = prescored.pop(ch, {})
            for h in range(NHL):
                ps_sum = psum.tile([P, CHUNK], F32, tag="ps", name="ps")
                ps_ot = psum.tile([P, CHUNK], F32, tag="ps", name="ps")
                nxt = pre.get(h, [])
                next_j = len(nxt)
                while len(nxt) < 2 and next_j < njt:
                    nxt.append(score(ch, h, next_j))
                    next_j += 1
                for j in range(njt):
                    pt, lo = nxt.pop(0)
                    if next_j < njt:
                        nxt.append(score(ch, h, next_j))
                        next_j += 1
                    nc.tensor.matmul(ps_sum[:, lo:], ones_sb[:], pt[:, lo:],
                                     start=(j == 0), stop=(j == njt - 1))
                    nc.tensor.matmul(
                        ps_ot[:, lo:], v_sb[j // 4][:, (j % 4) * P:(j % 4 + 1) * P],
                        pt[:, lo:], start=(j == 0), stop=(j == njt - 1))
                recip = tmppool.tile([P, CHUNK], F32, tag="recip", name="recip", bufs=2)
                nc.vector.reciprocal_approx_fast(out=recip[:], in_=ps_sum[:])
                nc.vector.tensor_mul(ot_sb[h][ch][:], ps_ot[:], recip[:])

        def phase_c(ch, pre_ch=None):
            """output projection for the 4 seq tiles of chunk ch.  The
            PSUM->SBUF bf16 copies alternate DVE/ACT so neither engine queues
            more than half of them ahead of the next B phase's exps.  pre_ch:
            emit the next B phase's first two scores+exps before the last
            quarter, hiding the exp latency at the B start."""
            for tq in range(4):
                if tq == 3 and pre_ch is not None:
                    prescored[pre_ch] = {0: [score(pre_ch, 0, 0),
                                             score(pre_ch, 0, 1)]}
                t = 4 * ch + tq
                for d in range(NDC):
                    ps_o = psum.tile([P, CHUNK], F32, tag="ps", name="ps")
                    for h in range(NHL):
                        nc.tensor.matmul(
                            ps_o[:], ot_sb[h][ch][:, tq * P:(tq + 1) * P],
                            wo_sb[:, h, d * CHUNK:(d + 1) * CHUNK],
                            start=(h == 0), stop=(h == NHL - 1))
                    ob = obpool.tile([P, CHUNK], BF16, tag="ob", name="ob")
                    if d % 2 == 0:
                        nc.vector.tensor_copy(out=ob[:], in_=ps_o[:])
                    else:
                        nc.scalar.copy(ob[:], ps_o[:])
                    oeng = nc.sync if d % 2 == 0 else nc.gpsimd
                    oeng.dma_start(
                        out[t * P:(t + 1) * P, d * CHUNK:(d + 1) * CHUNK], ob[:])

        # [A A C B] windows: C's PE work separates the A pair's ACT swap
        # copies (and C's own alternating copies) from B's exp stream, so by
        # the time the PE reaches B's matmuls ACT is drained and exps issue
        # back-to-back.  B0 (the one B with no C before it) would wait ~4.5us
        # for exps queued behind A3's swap copies, so most of its scores+exps
        # are emitted mid-A3 where ACT is idle (deferring A3's copies past B0
        # instead would deadlock the shared 8-bank PSUM ring).
        def pre_b0():
            prescored[0] = {
                0: [score(0, 0, j) for j in range(4)],
                1: [score(0, 1, j) for j in range(4)],
                2: [score(0, 2, j) for j in range(4)],
                3: [score(0, 3, j) for j in range(2)],
            }

        phase_a(0)
        phase_a(1)
        phase_a(2)
        phase_a(3, mid_cb=pre_b0)
        phase_b(0)
        for ch in range(2, NCH):
            phase_a(2 * ch)
            phase_a(2 * ch + 1)
            phase_c(ch - 2, pre_ch=ch - 1)
            phase_b(ch - 1)
        phase_c(NCH - 2, pre_ch=NCH - 1)
        phase_b(NCH - 1)
        phase_c(NCH - 1)

    return nc


def _make_in_maps(x, freqs_cis, wqkv, wo):
    scale = np.float32(1.0 / np.sqrt(HD))
    # chunk-major x: xG[p, ch, g, j, s] = x[0, ch*512+s, (4g+j)*128+p], so a
    # (ch, g) slice is one contiguous 4KB run per partition.
    xG = np.ascontiguousarray(
        np.asarray(x)[0].reshape(S // 512, 512, DIM // 512, 4, 128)
        .transpose(4, 0, 2, 3, 1)).astype(NPBF16)
    cos = freqs_cis[:, :, 0].T.astype(np.float32)        # (64, S)
    sin = freqs_cis[:, :, 1].T.astype(np.float32)
    cosT = np.ascontiguousarray(
        np.concatenate([cos, cos], 0)).astype(NPBF16)   # (128, S)
    sinT = np.ascontiguousarray(
        np.concatenate([-sin, sin], 0)).astype(NPBF16)
    ones = np.ones((128, 128), NPBF16)
    kp = np.arange(128)[:, None]
    qp = np.arange(128)[None, :]
    maskT = np.where(kp <= qp, 0.0, -1e30).astype(np.float32)

    in_maps = []
    for c in range(N_CORES):
        rows = [wqkv[128 * (NHL * c + h) + PERM] * scale for h in range(NHL)]
        rows.append(wqkv[NH * HD + 128 * c + PERM])
        wqkT = np.ascontiguousarray(np.concatenate(rows, 0).T).astype(NPBF16)
        wvT = np.ascontiguousarray(
            wqkv[(NH + NKV) * HD + 128 * c:(NH + NKV) * HD + 128 * (c + 1)].T
        ).astype(NPBF16)
        woT = np.ascontiguousarray(
            wo[:, 128 * NHL * c:128 * NHL * (c + 1)].T).astype(NPBF16)
        in_maps.append({
            "xG": xG, "wqkT": wqkT, "wvT": wvT, "woT": woT,
            "cosT": cosT, "sinT": sinT, "onesW": ones, "maskT": maskT,
        })
    return in_maps


def kernel(x, freqs_cis, wqkv, wo):
    x = np.asarray(x, dtype=np.float32)
    freqs_cis = np.asarray(freqs_cis, dtype=np.float32)
    wqkv = np.asarray(wqkv, dtype=np.float32)
    wo = np.asarray(wo, dtype=np.float32)

    in_maps = _make_in_maps(x, freqs_cis, wqkv, wo)
    nc = bacc.Bacc("TRN2", target_bir_lowering=False, debug=False,
                   num_devices=N_CORES)
    build_attention_kernel(nc, S=S, DIM=DIM)
    nc.compile()
    res = run_bass_kernel_spmd(nc, in_maps, core_ids=list(range(N_CORES)))

    acc = np.zeros((S, DIM), np.float32)
    for r in res.results:
        acc += np.asarray(r["out"]).astype(np.float32)
    return acc[None]
